# revision 1
# baseline (speedup 1.0000x reference)
"""Trainium2 Bass kernel for nn_Attention_89172110999574.

Strategy (8 NeuronCores, data parallel — 1 batch element per core):
  - x transposed on-chip via PE; QKV projections as matmuls.
  - Scores computed TRANSPOSED (ST[j,i] = k_j . q_i) so that softmax
    reduction rides the attn@V matmul: rhs is exp(ST), lhsT is [v | 1],
    giving the softmax denominator as an extra output row for free.
  - Relative-position bias handled with zero gathers: the bias matrix is
    block-Toeplitz (block (ri,rj) depends only on |ri-rj|), so a per-head
    strip table MS[(g,cj), u, ci] = E_h[|u-31-g|, |ci-cj|] is built once
    with a handful of strided DMAs; every score tile's bias is then a
    contiguous strided slice of MS added into PSUM via one identity
    matmul (scaled identity folds the 1/scale**2 factor).
  - exp() without max-subtraction (scores are ~N(0,1); |s|<~8 so exp is
    safe in fp32), gelu deferred to a single phase to avoid ACT
    table-set switches.
  - fp32r (full-rate fp32 PE mode) for all big matmuls.
"""

import os
import sys

import numpy as np

for _p in ("/opt/trn_rl_repo", "/root/.axon_site/_ro/trn_rl_repo"):
    if os.path.isdir(_p) and _p not in sys.path:
        sys.path.insert(0, _p)

import concourse.bass as bass
import concourse.tile as tile
from concourse import mybir
from concourse.bass_utils import run_bass_kernel_spmd
from concourse.masks import make_identity

N = 1024          # tokens per batch (32*32)
D = 256           # model dim
H = 8             # heads
DK = 32           # head dim (qk)
DV = 64           # head dim (v)
DOUT = 256        # output dim
NCORES = 8
FM = 32           # fmap
SCALE = float(DK) ** -0.5          # 1/sqrt(32)
BN_C = float(1.0 / np.sqrt(1.0 + 1e-5))
F32 = mybir.dt.float32
F32R = mybir.dt.float32r

USE_F32R = True
# matmul-operand dtype: float32r tiles (producers round); F32 fallback
MDT = F32R if USE_F32R else F32


def _r(ap):
    """matmul operands are already MDT-typed tiles."""
    return ap


def build_nc():
    nc = bass.Bass("TRN2", target_bir_lowering=False, debug=False)

    x = nc.dram_tensor("x", [N, D], F32, kind="ExternalInput").ap()
    wq = nc.dram_tensor("wq", [D, H * DK], F32, kind="ExternalInput").ap()
    wk = nc.dram_tensor("wk", [D, H * DK], F32, kind="ExternalInput").ap()
    wv = nc.dram_tensor("wv", [D, H * DV], F32, kind="ExternalInput").ap()
    wo = nc.dram_tensor("wo", [H * DV, DOUT], F32, kind="ExternalInput").ap()
    pe = nc.dram_tensor("pe", [N, H], F32, kind="ExternalInput").ap()
    bo = nc.dram_tensor("bo", [DOUT], F32, kind="ExternalInput").ap()
    gam = nc.dram_tensor("gam", [DOUT], F32, kind="ExternalInput").ap()
    bet = nc.dram_tensor("bet", [DOUT], F32, kind="ExternalInput").ap()
    out = nc.dram_tensor("out", [N, DOUT], F32, kind="ExternalOutput").ap()

    # scratch DRAM: per-head |s-31| expansion of pos_emb rows
    wfullh = nc.dram_tensor("wfullh", [H, 32, 63], F32R).ap()
    # scratch DRAM for partition-broadcasting the softmax recip rows
    rrd = nc.dram_tensor("rrd", [H, N], F32).ap()

    with tile.TileContext(nc) as tc:
        with (
            tc.tile_pool(name="const", bufs=1) as constp,
            tc.tile_pool(name="big", bufs=1) as bigp,
            tc.tile_pool(name="xin", bufs=3) as xinp,
            tc.tile_pool(name="exps", bufs=3) as expp,
            tc.tile_pool(name="small", bufs=2) as smallp,
            tc.tile_pool(name="yout", bufs=3) as youtp,
            tc.tile_pool(name="ps1", bufs=2, space="PSUM") as ps1p,
            tc.tile_pool(name="ps2", bufs=2, space="PSUM") as ps2p,
        ):
            # ---------------- constants / tables -----------------------
            ident = constp.tile([128, 128], F32)
            make_identity(nc, ident)
            isc = constp.tile([128, 128], MDT)
            # identity scaled by 1/scale^2 = 32: folds bias/scale into PSUM add
            nc.scalar.mul(isc, ident, float(DK))

            # Build wfull[h, t, s] = pos_emb[32*|t-31| + |s-31|, h].
            # 1) pos_emb -> SBUF E_sb[a, b, h] (contiguous)
            e_sb = smallp.tile([32, 32, 8], F32, tag="e_sb")
            nc.sync.dma_start(
                out=e_sb,
                in_=bass.AP(tensor=pe.tensor, offset=0,
                            ap=[[32 * H, 32], [H, 32], [1, 32 * H // 32]]),
            )
            # 2) s-flip on DVE: wrow[a, h, s] = E[a, |s-31|, h]
            wrow = smallp.tile([32, 8, 63], F32R, tag="wrow")
            nc.vector.tensor_copy(
                wrow[:, :, 0:31],
                bass.AP(tensor=e_sb.tensor, offset=e_sb.offset + 31 * 8,
                        ap=[e_sb.ap[0], [1, 8], [-8, 31]]),
            )
            nc.vector.tensor_copy(
                wrow[:, :, 31:63],
                bass.AP(tensor=e_sb.tensor, offset=e_sb.offset,
                        ap=[e_sb.ap[0], [1, 8], [8, 32]]),
            )
            # 3) dump wrow to DRAM: wfullh[h, a, s] = E_h[a, |s-31|]
            for h in range(H):
                nc.sync.dma_start(out=wfullh[h], in_=wrow[:, h, :])

            # 4) UWsb[cj, h, a, ci] = E_h[a, |ci-cj|] = wfullh[h, a, 31-cj+ci]
            #    one DMA per cj (all-positive strides, contiguous last dim)
            ms = bigp.tile([128, H, 66, 32], F32R)
            with tc.tile_pool(name="uw", bufs=1) as uwp:
                uwsb = uwp.tile([32, H, 32, 32], F32R)
                # alternate DMA queues to halve the gather wall time
                for cj in range(32):
                    eng = nc.sync if cj % 2 == 0 else nc.gpsimd
                    eng.dma_start(
                        out=uwsb[cj:cj + 1],
                        in_=bass.AP(tensor=wfullh.tensor, offset=31 - cj,
                                    ap=[[2016, 8], [63, 32], [1, 32]]),
                    )
                # 5) u-expansion into MS[(g,cj), h, u, ci] = E_h[|u-31-g|,|ci-cj|]
                #    upper half u=31+g..62+g plain; lower half u=g..30+g reads
                #    uwsb with a descending (negative mid-dim step is legal)
                # head-major order: head h's bias slices become ready after
                # its own 8 fills, letting phase C start ~8x earlier; gpsimd
                # queue keeps this off the sync-DMA queue
                for h in range(H):
                    for g in range(4):
                        nc.sync.dma_start(
                            out=ms[32 * g:32 * (g + 1), h, 31 + g:63 + g, :],
                            in_=uwsb[:, h, :, :],
                        )
                        nc.gpsimd.dma_start(
                            out=ms[32 * g:32 * (g + 1), h, g:31 + g, :],
                            in_=bass.AP(tensor=uwsb.tensor,
                                        offset=uwsb.offset + h * 1024 + 31 * 32,
                                        ap=[uwsb.ap[0], [-32, 31], [1, 32]]),
                        )
            # ---------------- weights ----------------------------------
            wq_sb = constp.tile([128, 2, 256], MDT)
            wk_sb = constp.tile([128, 2, 256], MDT)
            wv_sb = constp.tile([128, 2, 512], MDT)
            wo_sb = constp.tile([128, 4, 256], MDT)
            for dst_sb, wsrc, nk in ((wq_sb, wq, 2), (wk_sb, wk, 2),
                                     (wv_sb, wv, 2), (wo_sb, wo, 4)):
                for kt in range(nk):
                    wstg = xinp.tile([128, 512], F32, tag="wstg", bufs=2)
                    cols = dst_sb.shape[2]
                    nc.scalar.dma_start(out=wstg[:, 0:cols],
                                          in_=wsrc[128 * kt:128 * (kt + 1), :])
                    nc.vector.tensor_copy(dst_sb[:, kt, :], wstg[:, 0:cols])

            # BN affine rows (partition-broadcast straight from DRAM):
            # g2 = gamma*c ; b2 = bo*g2 + beta
            g2b = constp.tile([128, DOUT], F32)
            b2b = constp.tile([128, DOUT], F32)
            tmpb = constp.tile([128, DOUT], F32)
            nc.sync.dma_start(
                out=g2b, in_=bass.AP(tensor=gam.tensor, offset=0,
                                     ap=[[0, 128], [1, DOUT]]))
            nc.sync.dma_start(
                out=b2b, in_=bass.AP(tensor=bet.tensor, offset=0,
                                     ap=[[0, 128], [1, DOUT]]))
            nc.sync.dma_start(
                out=tmpb, in_=bass.AP(tensor=bo.tensor, offset=0,
                                      ap=[[0, 128], [1, DOUT]]))
            nc.scalar.mul(g2b, g2b, BN_C)
            nc.vector.tensor_mul(tmpb, tmpb, g2b)
            nc.vector.tensor_add(b2b, b2b, tmpb)

            # ---------------- phase A: x -> xT --------------------------
            xT = bigp.tile([128, 2, N], MDT)
            for nt in range(8):
                xa = xinp.tile([128, D], F32, tag="xa", bufs=3)
                nc.scalar.dma_start(out=xa, in_=x[128 * nt:128 * (nt + 1), :])
                for dt in range(2):
                    pst = ps1p.tile([128, 128], F32, tag="ps1")
                    nc.tensor.transpose(pst, xa[:, 128 * dt:128 * (dt + 1)], ident)
                    nc.vector.tensor_copy(xT[:, dt, 128 * nt:128 * (nt + 1)], pst)

            # ---------------- phase B: QKV proj -------------------------
            qT = bigp.tile([128, 2, N], MDT)
            kT = bigp.tile([128, 2, N], MDT)
            for dst_sb, w_sb in ((qT, wq_sb), (kT, wk_sb)):
                for mt in range(2):
                    for ic in range(2):
                        ps = ps1p.tile([128, 512], F32, tag="ps1")
                        for kt in range(2):
                            nc.tensor.matmul(
                                ps,
                                _r(w_sb[:, kt, 128 * mt:128 * (mt + 1)]),
                                _r(xT[:, kt, 512 * ic:512 * (ic + 1)]),
                                start=(kt == 0), stop=(kt == 1),
                            )
                        nc.vector.tensor_copy(dst_sb[:, mt, 512 * ic:512 * (ic + 1)], ps)

            # v, augmented with a ones column per head for the softmax
            # denominator: lhsT = [v | 1] -> denominator lands at out row 64.
            va = bigp.tile([128, 8, 8, 65], MDT)
            # ones columns only: ACT Copy with scale=0, bias=1 writes 1.0
            # into the 65th column of each head slot (the v copies fill the
            # rest); avoids a 40us whole-tile memset on the gpsimd engine
            nc.scalar.activation(va[:, :, :, 64:65],
                                 ident[:, 0:64],
                                 mybir.ActivationFunctionType.Copy,
                                 bias=1.0, scale=0.0)
            for jt in range(8):
                ps = ps1p.tile([128, 512], F32, tag="ps1")
                for kt in range(2):
                    nc.tensor.matmul(
                        ps,
                        _r(xT[:, kt, 128 * jt:128 * (jt + 1)]),
                        _r(wv_sb[:, kt, :]),
                        start=(kt == 0), stop=(kt == 1),
                    )
                psr = ps.rearrange("p (h v) -> p h v", v=64)
                nc.vector.tensor_copy(va[:, jt, :, 0:64], psr)

            # ---------------- phase C: attention ------------------------
            gT = bigp.tile([128, 4, N], MDT)
            for h in range(H):
                mtk = h // 4
                pb = 32 * (h % 4)
                po = ps1p.tile([128, 1024], F32, tag="ps1")
                es_prev = None
                for jt in range(9):
                    if jt < 8:
                        ps = ps2p.tile([128, 1024], F32, tag="st")
                        for ic in range(2):
                            nc.tensor.matmul(
                                ps[:, 512 * ic:512 * (ic + 1)],
                                _r(kT[pb:pb + 32, mtk, 128 * jt:128 * (jt + 1)]),
                                _r(qT[pb:pb + 32, mtk, 512 * ic:512 * (ic + 1)]),
                                start=True, stop=False,
                                tile_position=(pb, 0),
                            )
                            u0 = 16 * ic + 31 - 4 * jt
                            nc.tensor.matmul(
                                ps[:, 512 * ic:512 * (ic + 1)],
                                _r(isc),
                                ms[:, h, u0:u0 + 16, :],
                                start=False, stop=True,
                            )
                        es = expp.tile([128, 1024], MDT, tag="es")
                        nc.scalar.activation(es, ps,
                                             mybir.ActivationFunctionType.Exp,
                                             scale=SCALE)
                    # attnV one stage behind so PE never stalls on exp
                    if jt > 0:
                        for ic in range(2):
                            nc.tensor.matmul(
                                po[0:65, 512 * ic:512 * (ic + 1)],
                                _r(va[:, jt - 1, h, :]),
                                _r(es_prev[:, 512 * ic:512 * (ic + 1)]),
                                start=(jt == 1), stop=(jt == 8),
                            )
                    es_prev = es
                # normalize: out/denominator (DMA-broadcast the recip row
                # across partitions; engines cannot step-0 broadcast)
                rr = smallp.tile([1, N], F32, tag="rr", bufs=1)
                nc.vector.reciprocal(rr, po[64:65, :])
                nc.sync.dma_start(out=rrd[h, :].unsqueeze(0), in_=rr)
                rrb = smallp.tile([64, N], F32, tag="rrb", bufs=1)
                nc.sync.dma_start(
                    out=rrb,
                    in_=bass.AP(tensor=rrd.tensor, offset=h * N,
                                ap=[[0, 64], [1, N]]),
                )
                nc.vector.tensor_mul(
                    gT[64 * (h % 2):64 * (h % 2) + 64, h // 2, :],
                    po[0:64, :],
                    rrb,
                )

            # ---------------- phase D: gelu (one table switch, in-place) -
            for kt in range(4):
                nc.scalar.activation(gT[:, kt, :], gT[:, kt, :],
                                     mybir.ActivationFunctionType.Gelu)

            # ---------------- phase E: out proj + BN --------------------
            for it in range(8):
                ps = ps1p.tile([128, 512], F32, tag="ps1")
                for kt in range(4):
                    nc.tensor.matmul(
                        ps[:, 0:256],
                        _r(gT[:, kt, 128 * it:128 * (it + 1)]),
                        _r(wo_sb[:, kt, :]),
                        start=(kt == 0), stop=(kt == 3),
                    )
                yt = youtp.tile([128, DOUT], F32, tag="yt")
                nc.vector.tensor_mul(yt, ps[:, 0:256], g2b)
                nc.vector.tensor_add(yt, yt, b2b)
                nc.sync.dma_start(out=out[128 * it:128 * (it + 1), :], in_=yt)

    _split_excess_waits(nc)
    return nc


def _split_excess_waits(nc):
    """walrus rejects >1 sem-wait per instruction ("Too many sync wait
    commands"); unroll extras into a chain of single-wait same-engine
    NoOps directly before the instruction."""
    ctr = 0
    for fn in nc.m.functions:
        for blk in fn.blocks:
            out = []
            for inst in blk.instructions:
                si = inst.sync_info
                if si is not None and len(si.on_wait) > 1:
                    for w in si.on_wait[:-1]:
                        nop = mybir.InstNoOp(name=f"waitnop-{ctr}")
                        ctr += 1
                        nop.engine = inst.engine
                        nop.sync_info = mybir.SyncInfo(
                            on_wait=[w], on_update=[])
                        out.append(nop)
                    inst.sync_info = mybir.SyncInfo(
                        on_wait=[si.on_wait[-1]], on_update=list(si.on_update))
                out.append(inst)
            blk.instructions = out


_NC_CACHE = None


def kernel(**inputs) -> np.ndarray:
    global _NC_CACHE
    x = np.ascontiguousarray(inputs["x"], dtype=np.float32)        # (8,32,32,256)
    shared = {
        "wq": np.ascontiguousarray(inputs["Wq"], dtype=np.float32),
        "wk": np.ascontiguousarray(inputs["Wk"], dtype=np.float32),
        "wv": np.ascontiguousarray(inputs["Wv"], dtype=np.float32),
        "wo": np.ascontiguousarray(inputs["Wo"], dtype=np.float32),
        "pe": np.ascontiguousarray(inputs["pos_emb"], dtype=np.float32),
        "bo": np.ascontiguousarray(inputs["bo"], dtype=np.float32),
        "gam": np.ascontiguousarray(inputs["gamma"], dtype=np.float32),
        "bet": np.ascontiguousarray(inputs["beta"], dtype=np.float32),
    }
    in_maps = []
    for c in range(NCORES):
        m = dict(shared)
        m["x"] = np.ascontiguousarray(x[c].reshape(N, D))
        in_maps.append(m)

    if _NC_CACHE is None:
        _NC_CACHE = build_nc()
    res = run_bass_kernel_spmd(_NC_CACHE, in_maps, core_ids=list(range(NCORES)))
    outs = [res.results[c]["out"].reshape(FM, FM, DOUT) for c in range(NCORES)]
    return np.stack(outs, axis=0)


if __name__ == "__main__":
    build_nc()
    print("build ok")



# revision 5
# speedup vs baseline: 1.2595x; 1.2595x over previous
"""Trainium2 Bass kernel for nn_Attention_89172110999574.

Strategy (8 NeuronCores, data parallel — 1 batch element per core):
  - Scores computed TRANSPOSED (ST[j,i] = k_j . q_i), operands bf16.
  - attn@V swapped: lhsT = exp-scores slice [128 j, 128 i] (stationary),
    rhs = [v | 1] [128 j, 65] -> out OT[i-slice, dv|den] in PSUM. N=65 per
    matmul instead of 512 -> attnV PE cost drops ~4x; softmax denominator
    rides as output column 64.
  - Relative-position bias is block-Toeplitz. Two application paths balanced
    across engines:
      * type-A heads (NB..7): identity-matmul adds raw bias strips into the
        score PSUM (spare PE capacity).
      * type-B heads (0..NB-1): exp(bias) table multiplied into exp(scores)
        on DVE (exp(a+b) = exp(a)*exp(b)).
    Strip tables built by direct strided DRAM->SBUF DMAs (per-partition
    shifted windows of the |.|-mirrored row table).
  - Normalization: per-partition reciprocal of OT column 64 (DVE), applied
    by Pool tensor_scalar into og[i, is, h', dv]; gelu batched on ACT per
    head-pair; each head-pair block moved to phase-E layout by ONE hardware
    DMA-transpose (out[p,m,l] = in[l, 128m+p]).
  - Output projection + BatchNorm affine fused on DVE.
"""

import os
import sys

import numpy as np

for _p in ("/opt/trn_rl_repo", "/root/.axon_site/_ro/trn_rl_repo"):
    if os.path.isdir(_p) and _p not in sys.path:
        sys.path.insert(0, _p)

import concourse.bass as bass
import concourse.tile as tile
from concourse import mybir
from concourse.bass_utils import run_bass_kernel_spmd
from concourse.masks import make_identity

N = 1024          # tokens per batch (32*32)
D = 256           # model dim
H = 8             # heads
DK = 32           # head dim (qk)
DV = 64           # head dim (v)
DOUT = 256        # output dim
NCORES = 8
FM = 32           # fmap
SCALE = float(DK) ** -0.5          # 1/sqrt(32)
RS32 = float(np.sqrt(32.0))        # bias premultiplier: bias/scale = sqrt(32)*E
BN_C = float(1.0 / np.sqrt(1.0 + 1e-5))
F32 = mybir.dt.float32
F32R = mybir.dt.float32r
BF16 = mybir.dt.bfloat16

# heads 0..NB-1 use the exp(bias)-multiply path (DVE); heads NB..7 use
# PE identity-matmul bias adds. Balances PE vs ACT vs DVE busy time.
NB = 3
NA = H - NB
EXP_LAG = 2  # attnV trails scores by this many jt stages


def build_nc():
    nc = bass.Bass("TRN2", target_bir_lowering=False, debug=False)

    # x declared f32r (bit-identical to f32) so transposes run all-f32r
    x = nc.dram_tensor("x", [N, D], F32R, kind="ExternalInput").ap()
    wq = nc.dram_tensor("wq", [D, H * DK], F32, kind="ExternalInput").ap()
    wk = nc.dram_tensor("wk", [D, H * DK], F32, kind="ExternalInput").ap()
    wv = nc.dram_tensor("wv", [D, H * DV], F32, kind="ExternalInput").ap()
    wo = nc.dram_tensor("wo", [H * DV, DOUT], F32, kind="ExternalInput").ap()
    pe = nc.dram_tensor("pe", [N, H], F32, kind="ExternalInput").ap()
    bo = nc.dram_tensor("bo", [DOUT], F32, kind="ExternalInput").ap()
    gam = nc.dram_tensor("gam", [DOUT], F32, kind="ExternalInput").ap()
    bet = nc.dram_tensor("bet", [DOUT], F32, kind="ExternalInput").ap()
    out = nc.dram_tensor("out", [N, DOUT], F32, kind="ExternalOutput").ap()

    # scratch DRAM: per-head |s-31| expansion of pos_emb rows
    # wfa: raw values (type-A heads, f32r); wfb: exp(sqrt32*E) (type-B, bf16)
    wfa = nc.dram_tensor("wfa", [NA, 32, 63], F32R).ap()
    wfb = nc.dram_tensor("wfb", [NB, 32, 63], BF16).ap()

    with tile.TileContext(nc) as tc:
        with (
            tc.tile_pool(name="const", bufs=1) as constp,
            tc.tile_pool(name="big", bufs=1) as bigp,
            tc.tile_pool(name="xin", bufs=2) as xinp,
            tc.tile_pool(name="exps", bufs=4) as expp,
            tc.tile_pool(name="exps2", bufs=3) as exp2p,
            tc.tile_pool(name="small", bufs=1) as smallp,
            tc.tile_pool(name="yout", bufs=3) as youtp,
            tc.tile_pool(name="ps2", bufs=2, space="PSUM") as ps2p,
        ):
            # ---------------- constants / identity ----------------------
            ident = constp.tile([128, 128], F32)
            make_identity(nc, ident)
            identr = constp.tile([128, 128], F32R)
            nc.scalar.activation(identr, ident,
                                 mybir.ActivationFunctionType.Copy)

            # ---------------- pos-emb row tables ------------------------
            e_sb = smallp.tile([32, 32, 8], F32, tag="e_sb")
            nc.sync.dma_start(
                out=e_sb,
                in_=bass.AP(tensor=pe.tensor, offset=0,
                            ap=[[32 * H, 32], [H, 32], [1, 32 * H // 32]]),
            )
            # s-flip on DVE: wrow[a, h, s] = E[a, |s-31|, h]
            wrow = smallp.tile([32, 8, 63], F32R, tag="wrow")
            nc.vector.tensor_copy(
                wrow[:, :, 0:31],
                bass.AP(tensor=e_sb.tensor, offset=e_sb.offset + 31 * 8,
                        ap=[e_sb.ap[0], [1, 8], [-8, 31]]),
            )
            nc.vector.tensor_copy(
                wrow[:, :, 31:63],
                bass.AP(tensor=e_sb.tensor, offset=e_sb.offset,
                        ap=[e_sb.ap[0], [1, 8], [8, 32]]),
            )
            # exp'd rows for type-B heads (bf16): ewrow = exp(sqrt32 * E)
            ewrow = smallp.tile([32, NB, 63], BF16, tag="ewrow")
            nc.scalar.activation(ewrow, wrow[:, 0:NB, :],
                                 mybir.ActivationFunctionType.Exp,
                                 scale=RS32)
            # dump row tables to DRAM (scalar queue; sync stays unblocked)
            nc.scalar.dma_start(out=wfb, in_=ewrow)
            nc.scalar.dma_start(out=wfa, in_=wrow[:, NB:H, :])

            # strip tables, built per (head, g, half) directly from DRAM:
            #   table[32g+cj, h, u, ci] = row_h[|u-31-g|, |ci-cj|]
            emB = bigp.tile([128, NB, 66, 32], BF16)
            msA = bigp.tile([128, NA, 66, 32], F32R)

            def fill_strip(dst, src_t, h, engs):
                hbase = 2016 * h
                # ascending halves first: jt=0 slices need only u >= 31
                for g in range(4):
                    engs[g % len(engs)].dma_start(
                        out=dst[32 * g:32 * (g + 1), h, 31 + g:63 + g, :],
                        in_=bass.AP(tensor=src_t, offset=hbase + 31,
                                    ap=[[-1, 32], [63, 32], [1, 32]]),
                    )
                for g in range(4):
                    engs[(g + 1) % len(engs)].dma_start(
                        out=dst[32 * g:32 * (g + 1), h, g:31 + g, :],
                        in_=bass.AP(tensor=src_t,
                                    offset=hbase + 31 * 63 + 31,
                                    ap=[[-1, 32], [-63, 31], [1, 32]]),
                    )

            for hb in range(NB):
                fill_strip(emB, wfb.tensor, hb,
                           [nc.sync, nc.scalar] if hb == 0 else [nc.sync])
            for ha in range(NA):
                fill_strip(msA, wfa.tensor, ha, [nc.sync])

            # ---------------- weights (sync queue) ----------------------
            wq_sb = constp.tile([128, 2, 256], BF16)
            wk_sb = constp.tile([128, 2, 256], BF16)
            wv_sb = constp.tile([128, 2, 512], BF16)
            wo_sb = constp.tile([128, 4, 256], BF16)
            for i, (dst_sb, wsrc, nk, cols) in enumerate(
                    ((wq_sb, wq, 2, 256), (wk_sb, wk, 2, 256),
                     (wv_sb, wv, 2, 512), (wo_sb, wo, 4, 256))):
                wstg = xinp.tile([128, nk, cols], F32, tag=f"wstg{i}", bufs=1)
                nc.sync.dma_start(
                    out=wstg,
                    in_=bass.AP(tensor=wsrc.tensor, offset=0,
                                ap=[[cols, 128], [128 * cols, nk], [1, cols]]))
                nc.vector.tensor_copy(dst_sb, wstg)

            # BN affine rows (partition-broadcast straight from DRAM):
            # g2 = gamma*c ; b2 = bo*g2 + beta
            g2b = constp.tile([128, DOUT], F32)
            b2b = constp.tile([128, DOUT], F32)
            tmpb = constp.tile([128, DOUT], F32)
            nc.sync.dma_start(
                out=g2b, in_=bass.AP(tensor=gam.tensor, offset=0,
                                     ap=[[0, 128], [1, DOUT]]))
            nc.sync.dma_start(
                out=b2b, in_=bass.AP(tensor=bet.tensor, offset=0,
                                     ap=[[0, 128], [1, DOUT]]))
            nc.sync.dma_start(
                out=tmpb, in_=bass.AP(tensor=bo.tensor, offset=0,
                                      ap=[[0, 128], [1, DOUT]]))
            nc.scalar.mul(g2b, g2b, BN_C)
            nc.vector.tensor_mul(tmpb, tmpb, g2b)
            nc.vector.tensor_add(b2b, b2b, tmpb)

            # v augmented with a ones column per head for the softmax
            # denominator (becomes OT column 64)
            va = bigp.tile([128, 8, 8, 65], BF16)
            nc.scalar.activation(va[:, :, :, 64:65],
                                 ident[:, 0:64],
                                 mybir.ActivationFunctionType.Copy,
                                 bias=1.0, scale=0.0)

            xT = bigp.tile([128, 2, N], BF16)
            qT = bigp.tile([128, 2, N], BF16)
            kT = bigp.tile([128, 2, N], BF16)

            with tc.tile_pool(name="psAB", bufs=1, space="PSUM") as psABp:
                # ------------- phase A: x -> xT -------------------------
                for half in range(2):
                    xt_in = xinp.tile([128, 4, D], F32R, tag=f"xa{half}",
                                      bufs=1)
                    nc.scalar.dma_start(
                        out=xt_in,
                        in_=bass.AP(tensor=x.tensor, offset=half * 512 * D,
                                    ap=[[D, 128], [128 * D, 4], [1, D]]))
                    for sub in range(4):
                        nt = 4 * half + sub
                        for dt_ in range(2):
                            pst = psABp.tile([128, 128], F32R, tag="psab",
                                             bufs=2)
                            nc.tensor.transpose(
                                pst,
                                xt_in[:, sub, 128 * dt_:128 * (dt_ + 1)],
                                identr)
                            nc.vector.tensor_copy(
                                xT[:, dt_, 128 * nt:128 * (nt + 1)], pst)

                # ------------- phase B: QKV proj ------------------------
                for dst_sb, w_sb in ((qT, wq_sb), (kT, wk_sb)):
                    for mt in range(2):
                        for ic in range(2):
                            ps = psABp.tile([128, 512], F32, tag="psb",
                                            bufs=2)
                            for kt in range(2):
                                nc.tensor.matmul(
                                    ps,
                                    w_sb[:, kt, 128 * mt:128 * (mt + 1)],
                                    xT[:, kt, 512 * ic:512 * (ic + 1)],
                                    start=(kt == 0), stop=(kt == 1),
                                )
                            nc.vector.tensor_copy(
                                dst_sb[:, mt, 512 * ic:512 * (ic + 1)], ps)
                for jt in range(8):
                    ps = psABp.tile([128, 512], F32, tag="psb", bufs=2)
                    for kt in range(2):
                        nc.tensor.matmul(
                            ps,
                            xT[:, kt, 128 * jt:128 * (jt + 1)],
                            wv_sb[:, kt, :],
                            start=(kt == 0), stop=(kt == 1),
                        )
                    psr = ps.rearrange("p (h v) -> p h v", v=64)
                    nc.vector.tensor_copy(va[:, jt, :, 0:64], psr)

            # identity scaled by 32 = 1/scale^2: folds bias/scale into the
            # PSUM bias add (emitted late; only type-A heads use it)
            isc = constp.tile([128, 128], F32R)
            nc.scalar.mul(isc, identr, float(DK))

            # ---------------- phase C: attention ------------------------
            # og per head-pair: [128 i-low, 8 is, 2 h', 64 dv] bf16; after
            # gelu, ONE dma transpose -> gT band [128 hv, 8 is, 128 i-low]
            ogt = []
            gtb = []
            for p in range(4):
                og_p = bigp.tile([128, 8, 2, 64], BF16, tag=f"og{p}",
                                 name=f"og{p}")
                gt_p = bigp.tile([128, 8, 128], BF16, tag=f"gt{p}",
                                 name=f"gt{p}")
                ogt.append(og_p)
                gtb.append(gt_p)
            rd = constp.tile([128, 8, 8], F32)  # reciprocal denominators

            with tc.tile_pool(name="otp", bufs=4, space="PSUM") as otp:
                for h in range(H):
                    typeB = h < NB
                    mtk = h // 4
                    pb = 32 * (h % 4)
                    pr = h // 2
                    hq = h % 2
                    ota = otp.tile([128, 4, 65], F32, tag="ot")
                    otb = otp.tile([128, 4, 65], F32, tag="ot")
                    es_q = [None] * 8   # attnV operand per jt (es or es2)
                    for jt in range(8 + EXP_LAG):
                        if jt < 8:
                            ps = ps2p.tile([128, 1024], F32, tag="st")
                            for ic in range(2):
                                nc.tensor.matmul(
                                    ps[:, 512 * ic:512 * (ic + 1)],
                                    kT[pb:pb + 32, mtk,
                                       128 * jt:128 * (jt + 1)],
                                    qT[pb:pb + 32, mtk,
                                       512 * ic:512 * (ic + 1)],
                                    start=True, stop=typeB,
                                    tile_position=(pb, 0),
                                )
                                if not typeB:
                                    u0 = 16 * ic + 31 - 4 * jt
                                    nc.tensor.matmul(
                                        ps[:, 512 * ic:512 * (ic + 1)],
                                        isc,
                                        msA[:, h - NB, u0:u0 + 16, :],
                                        start=False, stop=True,
                                    )
                            es = expp.tile([128, 1024], BF16, tag="es")
                            nc.scalar.activation(
                                es, ps, mybir.ActivationFunctionType.Exp,
                                scale=SCALE)
                            if typeB:
                                es2 = exp2p.tile([128, 32, 32], BF16,
                                                 tag="es2")
                                nc.vector.tensor_mul(
                                    es2,
                                    es.rearrange("p (a b) -> p a b", b=32),
                                    emB[:, h, 31 - 4 * jt:63 - 4 * jt, :],
                                )
                                es_q[jt] = es2.rearrange("p a b -> p (a b)")
                            else:
                                es_q[jt] = es
                        # attnV trails by EXP_LAG stages so PE never stalls
                        jv = jt - EXP_LAG
                        if jv >= 0:
                            esv = es_q[jv]
                            for isl in range(8):
                                ot = ota if isl < 4 else otb
                                nc.tensor.matmul(
                                    ot[:, isl % 4, :],
                                    esv[:, 128 * isl:128 * (isl + 1)],
                                    va[:, jv, h, :],
                                    start=(jv == 0), stop=(jv == 7),
                                )
                    # drain head: reciprocal of denominators, normalize on
                    # Pool into og, per-pair gelu + dma-transpose
                    nc.vector.reciprocal(
                        rd[:, h, 0:4],
                        ota[:, :, 64:65].rearrange("p a b -> p (a b)"))
                    nc.vector.reciprocal(
                        rd[:, h, 4:8],
                        otb[:, :, 64:65].rearrange("p a b -> p (a b)"))
                    for isl in range(8):
                        ot = ota if isl < 4 else otb
                        nc.gpsimd.tensor_scalar_mul(
                            ogt[pr][:, isl, hq, :],
                            ot[:, isl % 4, 0:64],
                            rd[:, h, isl:isl + 1],
                        )
                    if hq == 1:
                        og2 = ogt[pr].rearrange("p a b c -> p (a b c)")
                        nc.scalar.activation(
                            og2, og2, mybir.ActivationFunctionType.Gelu)
                        nc.sync.dma_start_transpose(gtb[pr], og2)

                # ------------- phase E: out proj + BN -------------------
                for it in range(8):
                    ps = ps2p.tile([128, 1024], F32, tag="st")
                    for kt in range(4):
                        nc.tensor.matmul(
                            ps[:, 0:256],
                            gtb[kt][:, it, :],
                            wo_sb[:, kt, :],
                            start=(kt == 0), stop=(kt == 3),
                        )
                    yt = youtp.tile([128, DOUT], F32, tag="yt")
                    nc.vector.tensor_mul(yt, ps[:, 0:256], g2b)
                    nc.vector.tensor_add(yt, yt, b2b)
                    nc.sync.dma_start(out=out[128 * it:128 * (it + 1), :],
                                      in_=yt)

    _split_excess_waits(nc)
    return nc


def _split_excess_waits(nc):
    """walrus rejects >1 sem-wait per instruction ("Too many sync wait
    commands"); unroll extras into a chain of single-wait same-engine
    NoOps directly before the instruction."""
    ctr = 0
    for fn in nc.m.functions:
        for blk in fn.blocks:
            out = []
            for inst in blk.instructions:
                si = inst.sync_info
                if si is not None and len(si.on_wait) > 1:
                    for w in si.on_wait[:-1]:
                        nop = mybir.InstNoOp(name=f"waitnop-{ctr}")
                        ctr += 1
                        nop.engine = inst.engine
                        nop.sync_info = mybir.SyncInfo(
                            on_wait=[w], on_update=[])
                        out.append(nop)
                    inst.sync_info = mybir.SyncInfo(
                        on_wait=[si.on_wait[-1]], on_update=list(si.on_update))
                out.append(inst)
            blk.instructions = out


_NC_CACHE = None


def kernel(**inputs) -> np.ndarray:
    global _NC_CACHE
    x = np.ascontiguousarray(inputs["x"], dtype=np.float32)        # (8,32,32,256)
    shared = {
        "wq": np.ascontiguousarray(inputs["Wq"], dtype=np.float32),
        "wk": np.ascontiguousarray(inputs["Wk"], dtype=np.float32),
        "wv": np.ascontiguousarray(inputs["Wv"], dtype=np.float32),
        "wo": np.ascontiguousarray(inputs["Wo"], dtype=np.float32),
        "pe": np.ascontiguousarray(inputs["pos_emb"], dtype=np.float32),
        "bo": np.ascontiguousarray(inputs["bo"], dtype=np.float32),
        "gam": np.ascontiguousarray(inputs["gamma"], dtype=np.float32),
        "bet": np.ascontiguousarray(inputs["beta"], dtype=np.float32),
    }
    in_maps = []
    for c in range(NCORES):
        m = dict(shared)
        m["x"] = np.ascontiguousarray(x[c].reshape(N, D))
        in_maps.append(m)

    if _NC_CACHE is None:
        _NC_CACHE = build_nc()
    res = run_bass_kernel_spmd(_NC_CACHE, in_maps, core_ids=list(range(NCORES)))
    outs = [res.results[c]["out"].reshape(FM, FM, DOUT) for c in range(NCORES)]
    return np.stack(outs, axis=0)


if __name__ == "__main__":
    nc = build_nc()
    print("build ok")
    from concourse.timeline_sim import TimelineSim
    tl = TimelineSim(nc, trace=False)
    tl.simulate()
    print(f"HW exec time: {tl.time:.0f} ns")
